# revision 7
# baseline (speedup 1.0000x reference)
"""Multi-head attention Trainium2 kernel (8 NeuronCores, SPMD).

Problem: B=2, S=2048, d_model=1024, H=16 heads, dk=64.
    q = Q@WQ_h, k = K@WK_h, v = V@WV_h  (per head)
    scores = q k^T / sqrt(dk) + mask;  attn = softmax(scores)
    out = concat_h(attn @ v) @ WO

Sharding: 8 cores = 2 batches x 4 head-groups (4 heads each).  Each core
computes a full [S, d_model] partial output (its heads' contribution through
WO); host sums the 4 partials per batch.

Per-core dataflow (all matmul inputs bf16, accumulation f32):
  - host supplies transposed activations X^T [D, S] so projections contract
    over d with natural layouts
  - q/k projected into [dk, S] layout (head pairs stacked -> full 128-wide
    matmuls); v projected into [S, dk] layout with an appended ones column
  - scores computed transposed: S^T[k, q] = k q^T (contraction dk=64, two
    heads row-packed on the PE)
  - attn_unnorm^T = exp(S^T) * exp(mask)^T  (exp on ScalarE PSUM->SBUF bf16;
    the mask-add becomes a bf16 2x-rate multiply on VectorE; scores are
    O(+-8) so unnormalized exp is safe in f32/bf16)
  - PV: O^T[dk+1, q] = [v | 1]^T @ attn^T -- the ones column makes the
    softmax denominator Z[q] ride along as row 64
  - normalize O^T rows by 1/Z during PSUM eviction (reciprocal + DMA
    partition-broadcast of 1/Z)
  - WO: partial[q, n] accumulates lhsT = stacked O^T head-pair chunks
"""

import os

import numpy as np
import ml_dtypes

import concourse.bass as bass
import concourse.tile as tile
import concourse.mybir as mybir
from concourse import bacc
from concourse.bass_utils import run_bass_kernel_spmd

BF16 = mybir.dt.bfloat16
F32 = mybir.dt.float32

B = 2
S = 2048
D = 1024
H = 16
DK = 64
N_CORES = 8
HPC = H // (N_CORES // B)  # heads per core = 4
P = 128

NB_F = np.dtype(ml_dtypes.bfloat16)

# stash for test harness
LAST_RESULTS = None


def _build_program():
    nc = bacc.Bacc("TRN2", target_bir_lowering=False, debug=False)

    qT = nc.dram_tensor("qT", [D, S], BF16, kind="ExternalInput")
    kT = nc.dram_tensor("kT", [D, S], BF16, kind="ExternalInput")
    vT = nc.dram_tensor("vT", [D, S], BF16, kind="ExternalInput")
    eT = nc.dram_tensor("eT", [S, S], BF16, kind="ExternalInput")  # exp(mask)^T
    wq = nc.dram_tensor("wq", [D, HPC * DK], BF16, kind="ExternalInput")
    wk = nc.dram_tensor("wk", [D, HPC * DK], BF16, kind="ExternalInput")
    wv = nc.dram_tensor("wv", [D, HPC * DK], BF16, kind="ExternalInput")
    wo = nc.dram_tensor("wo", [HPC * DK, D], BF16, kind="ExternalInput")
    out = nc.dram_tensor("out", [S, D], F32, kind="ExternalOutput")

    ND = D // P          # 8 d chunks
    NK = S // P          # 16 k chunks
    NQ = S // 512        # 4 q blocks of 512
    NPAIR = HPC // 2     # 2 head pairs

    with tile.TileContext(nc) as tc:
        with tc.tile_pool(name="persist", bufs=1) as persist:
            # persistent SBUF tensors
            w_sb = {}
            for name, t in (("wq", wq), ("wk", wk), ("wv", wv)):
                w_sb[name] = persist.tile([P, ND, HPC * DK], BF16, tag=f"w_{name}", name=f"w_{name}")
                nc.sync.dma_start(
                    w_sb[name],
                    t.rearrange("(dc p) m -> p dc m", p=P),
                )
            wo_sb = persist.tile([P, NPAIR, D], BF16, tag="wo")
            nc.sync.dma_start(wo_sb, wo.rearrange("(pr p) n -> p pr n", p=P))

            # projected activations
            qT_sb = persist.tile([P, NPAIR, S], BF16, tag="qT_sb")
            kT_sb = persist.tile([P, NPAIR, S], BF16, tag="kT_sb")
            # v with ones column: [s-part, kchunk, head, 65]
            v_sb = persist.tile([P, NK, HPC, DK + 1], BF16, tag="v_sb")
            nc.vector.memset(v_sb[:, :, :, DK : DK + 1], 1.0)

            # ---------------- phase 1: projections ----------------
            with (
                tc.tile_pool(name="xT", bufs=1) as xT_pool,
                tc.tile_pool(name="ps_proj", bufs=2, space="PSUM") as ps_proj,
                tc.tile_pool(name="ps_vproj", bufs=2, space="PSUM") as ps_vproj,
            ):
                xT_sb = {}
                for name, t in (("qT", qT), ("kT", kT), ("vT", vT)):
                    xT_sb[name] = xT_pool.tile([P, ND, S], BF16, tag=f"xT_{name}", name=f"xT_{name}")
                    nc.sync.dma_start(
                        xT_sb[name], t.rearrange("(dc p) s -> p dc s", p=P)
                    )

                # q/k projections: out[pair-dims(128), s] ; lhsT = w chunk
                for dst, wname, xname in (
                    (qT_sb, "wq", "qT"),
                    (kT_sb, "wk", "kT"),
                ):
                    for pr in range(NPAIR):
                        for sb in range(NQ):
                            ps = ps_proj.tile([P, 512], F32, tag="ps_proj")
                            for dc in range(ND):
                                nc.tensor.matmul(
                                    ps,
                                    w_sb[wname][:, dc, pr * P : (pr + 1) * P],
                                    xT_sb[xname][:, dc, sb * 512 : (sb + 1) * 512],
                                    start=(dc == 0),
                                    stop=(dc == ND - 1),
                                )
                            nc.vector.tensor_copy(
                                dst[:, pr, sb * 512 : (sb + 1) * 512], ps
                            )

                # v projection: out[s-chunk(128), 4*64] ; lhsT = vT chunk
                for kc in range(NK):
                    ps = ps_vproj.tile([P, HPC * DK], F32, tag="ps_vproj")
                    for dc in range(ND):
                        nc.tensor.matmul(
                            ps,
                            xT_sb["vT"][:, dc, kc * P : (kc + 1) * P],
                            w_sb["wv"][:, dc, :],
                            start=(dc == 0),
                            stop=(dc == ND - 1),
                        )
                    nc.vector.tensor_copy(
                        v_sb[:, kc, :, 0:DK],
                        ps.rearrange("p (h j) -> p h j", h=HPC),
                    )

            # ---------------- phase 2: attention + output ----------------
            with (
                tc.tile_pool(name="eT_pool", bufs=2) as eT_pool,
                tc.tile_pool(name="attn", bufs=2) as attn_pool,
                tc.tile_pool(name="es", bufs=3) as es_pool,
                tc.tile_pool(name="oT", bufs=3) as oT_pool,
                tc.tile_pool(name="rz", bufs=3) as rz_pool,
                tc.tile_pool(name="rzb", bufs=3) as rzb_pool,
                tc.tile_pool(name="outsb", bufs=3) as outsb_pool,
                tc.tile_pool(name="ps_s", bufs=2, space="PSUM") as ps_s_pool,
                tc.tile_pool(name="ps_pv", bufs=2, space="PSUM") as ps_pv_pool,
                tc.tile_pool(name="ps_wo", bufs=2, space="PSUM") as ps_wo_pool,
            ):
                for qb in range(NQ):
                    qs = slice(qb * 512, (qb + 1) * 512)
                    eT_blk = eT_pool.tile([P, NK, 512], BF16, tag="eT_blk")
                    nc.sync.dma_start(
                        eT_blk, eT[:, qs].rearrange("(kc p) q -> p kc q", p=P)
                    )
                    oT_pair_sb = []
                    for pr in range(NPAIR):
                        attnT = [
                            attn_pool.tile(
                                [P, NK, 512], BF16,
                                tag=f"attnT{hh}", name=f"attnT{hh}",
                            )
                            for hh in range(2)
                        ]
                        # scores + exp + mask-multiply, groups of 2 k-chunks
                        for kg in range(NK // 2):
                            for hh in range(2):
                                hb = hh * DK  # partition base of this head
                                ps_sc = ps_s_pool.tile(
                                    [P, 2, 512], F32, tag="ps_s", name="ps_sc"
                                )
                                for i in range(2):
                                    kc = kg * 2 + i
                                    nc.tensor.matmul(
                                        ps_sc[:, i, :],
                                        kT_sb[hb : hb + DK, pr, kc * P : (kc + 1) * P],
                                        qT_sb[hb : hb + DK, pr, qs],
                                        start=True,
                                        stop=True,
                                    )
                                es = es_pool.tile([P, 2, 512], BF16, tag="es")
                                nc.scalar.activation(
                                    es, ps_sc, mybir.ActivationFunctionType.Exp
                                )
                                nc.vector.tensor_mul(
                                    attnT[hh][:, kg * 2 : kg * 2 + 2, :],
                                    es,
                                    eT_blk[:, kg * 2 : kg * 2 + 2, :],
                                )
                        # PV per head
                        oT_sb = oT_pool.tile([P, 512], BF16, tag="oT_sb")
                        oT_pair_sb.append(oT_sb)
                        for hh in range(2):
                            h = pr * 2 + hh
                            ps_o = ps_pv_pool.tile([DK + 1, 512], F32, tag="ps_pv")
                            for kc in range(NK):
                                nc.tensor.matmul(
                                    ps_o,
                                    v_sb[:, kc, h, :],
                                    attnT[hh][:, kc, :],
                                    start=(kc == 0),
                                    stop=(kc == NK - 1),
                                )
                            rz = rz_pool.tile([1, 512], F32, tag="rz")
                            nc.vector.reciprocal(rz, ps_o[DK : DK + 1, :])
                            rzb = rzb_pool.tile([DK, 512], F32, tag="rzb")
                            nc.gpsimd.partition_broadcast(rzb, rz)
                            nc.vector.tensor_mul(
                                oT_sb[hh * DK : (hh + 1) * DK, :],
                                ps_o[0:DK, :],
                                rzb,
                            )
                    # WO for this q-block
                    for qq in range(4):
                        row0 = qb * 512 + qq * P
                        for nb in range(2):
                            ps_w = ps_wo_pool.tile([P, 512], F32, tag="ps_wo")
                            for pr in range(NPAIR):
                                nc.tensor.matmul(
                                    ps_w,
                                    oT_pair_sb[pr][:, qq * P : (qq + 1) * P],
                                    wo_sb[:, pr, nb * 512 : (nb + 1) * 512],
                                    start=(pr == 0),
                                    stop=(pr == NPAIR - 1),
                                )
                            osb = outsb_pool.tile([P, 512], F32, tag="osb")
                            nc.vector.tensor_copy(osb, ps_w)
                            nc.sync.dma_start(
                                out[row0 : row0 + P, nb * 512 : (nb + 1) * 512],
                                osb,
                            )

    nc.compile()
    return nc


_PROGRAM = None


def _get_program():
    global _PROGRAM
    if _PROGRAM is None:
        _PROGRAM = _build_program()
    return _PROGRAM


def prepare_in_maps(Q, K, V, additive_mask, WQ, WK, WV, WO):
    Q = np.asarray(Q, np.float32)
    K = np.asarray(K, np.float32)
    V = np.asarray(V, np.float32)
    mask = np.asarray(additive_mask, np.float32)
    WQ = np.asarray(WQ, np.float32)
    WK = np.asarray(WK, np.float32)
    WV = np.asarray(WV, np.float32)
    WO = np.asarray(WO, np.float32)

    # host prep
    scale = 1.0 / np.sqrt(DK)
    # stacked weights [D, H*DK], head-major columns; fold scale into WQ
    wq_all = np.ascontiguousarray((WQ * scale).transpose(1, 0, 2).reshape(D, H * DK))
    wk_all = np.ascontiguousarray(WK.transpose(1, 0, 2).reshape(D, H * DK))
    wv_all = np.ascontiguousarray(WV.transpose(1, 0, 2).reshape(D, H * DK))
    eT = np.ascontiguousarray(np.exp(mask).T).astype(NB_F)
    xT = {}
    for b in range(B):
        xT[("q", b)] = np.ascontiguousarray(Q[b].T).astype(NB_F)
        xT[("k", b)] = np.ascontiguousarray(K[b].T).astype(NB_F)
        xT[("v", b)] = np.ascontiguousarray(V[b].T).astype(NB_F)

    in_maps = []
    for c in range(N_CORES):
        b, g = divmod(c, N_CORES // B)
        hs = slice(g * HPC * DK, (g + 1) * HPC * DK)
        in_maps.append(
            {
                "qT": xT[("q", b)],
                "kT": xT[("k", b)],
                "vT": xT[("v", b)],
                "eT": eT,
                "wq": np.ascontiguousarray(wq_all[:, hs]).astype(NB_F),
                "wk": np.ascontiguousarray(wk_all[:, hs]).astype(NB_F),
                "wv": np.ascontiguousarray(wv_all[:, hs]).astype(NB_F),
                "wo": np.ascontiguousarray(WO[hs, :]).astype(NB_F),
                "out": np.zeros((S, D), np.float32),
            }
        )
    # "out" entries are outputs; run_bass_kernel_spmd builds its own out maps
    for m in in_maps:
        m.pop("out")
    return in_maps


def kernel(Q, K, V, additive_mask, key_padding_mask, WQ, WK, WV, WO):
    global LAST_RESULTS
    in_maps = prepare_in_maps(Q, K, V, additive_mask, WQ, WK, WV, WO)
    nc = _get_program()
    res = run_bass_kernel_spmd(
        nc,
        in_maps,
        core_ids=list(range(N_CORES)),
        trace=False,
    )
    LAST_RESULTS = res

    full = np.zeros((B, S, D), np.float32)
    for c in range(N_CORES):
        b = c // (N_CORES // B)
        full[b] += res.results[c]["out"]
    return full
